# revision 25
# baseline (speedup 1.0000x reference)
"""Cox proportional-hazards negative partial log-likelihood on 8 Trainium2
NeuronCores.

reference:
    risk_mask[i, j] = (time[j] >= time[i])
    risk_sum[i]     = sum_j exp(hazard[j]) * risk_mask[i, j]
    loss            = -mean((hazard - log(risk_sum)) * censor)

Because the risk set {j : time_j >= time_i} is a prefix of the
descending-time order, the O(N^2) masked reduction collapses to a prefix
sum: with hazard sorted by time descending,

    S[k]        = sum_{k' <= k} exp(hazard_sorted[k'])
    risk_sum[i] = S[cnt_i - 1],   cnt_i = |{j : time_j >= time_i}|

which is exact under ties (every tie of time_i sits inside the prefix).

Split of work:
  * device (each core): the O(N^2)-collapsed FP reduction — exp(hazard)
    on ACT, then the 8192-long inclusive prefix scan via the DVE
    TensorTensorScanArith recurrence on a [128 partitions x 64] layout
    (k = p*64 + t). One input DMA (fp16 [128, 64]) and one output DMA
    (bf16 [128, 64]) — the kernel is pure stream-through, which is the
    memory-regime shape this problem targets.
  * host: index bookkeeping plus the final O(N) merge — argsort by time,
    searchsorted for cnt_i, the 128-way exclusive cumsum of the
    partition totals (sc[:, -1], the scan's last column) to splice the
    per-partition prefixes into the global S, then log / gather /
    censored mean. This mirrors the previous masked-matmul kernel's host
    role (rank relabeling via np.unique, 8-way partial-sum merge, log,
    mean).
  * sharding: per-core work is O(N) = 16KB streamed, far below the cost
    of any cross-core collective, so the scan is replicated on all 8
    cores (SPMD requires a single program; output-range sharding would
    need per-core programs) and core 0's output is used.
"""

import numpy as np

N = 8192
P = 128
NT = N // P          # 64 elements per partition
NCORES = 8

_CACHE: dict = {}


def _ensure_path():
    try:
        import concourse.bass  # noqa: F401
    except ImportError:
        import sys

        sys.path.insert(0, "/opt/trn_rl_repo")


def _build_program():
    import concourse.bass as bass
    import concourse.mybir as mybir
    from concourse import tile

    f32 = mybir.dt.float32
    f16 = mybir.dt.float16
    bf16 = mybir.dt.bfloat16
    Alu = mybir.AluOpType
    Act = mybir.ActivationFunctionType

    nc = bass.Bass()
    # hazard sorted by time descending, reshaped [128, 64] (k = p*64 + t)
    hs = nc.declare_dram_parameter("hs", [P, NT], f16, isOutput=False)
    # per-partition inclusive prefix sums of exp(hazard)
    sc_out = nc.declare_dram_parameter("sc", [P, NT], bf16, isOutput=True)

    with tile.TileContext(nc) as tc:
        with tc.tile_pool(name="sb", bufs=1) as sb:
            h = sb.tile([P, NT], f16)
            nc.sync.dma_start(h[:], hs[:])
            z = sb.tile([P, NT], f32)
            nc.vector.memset(z[:], 0.0)

            # bias rides the z tile (already zeros) instead of the float
            # default: a float bias forces a const-AP, and with no const-AP
            # users the Bass prologue's four const memsets become dead
            # stores that _strip_dead_consts removes (they serialize on
            # Pool ahead of the start barrier, delaying the input DMA)
            e = sb.tile([P, NT], f32)
            nc.scalar.activation(e[:], h[:], Act.Exp, bias=z[:, 0:1])

            # sc[p, t] = sum_{t' <= t} e[p, t']  (DVE recurrence); the last
            # column carries the per-partition row total for the host-side
            # 128-way carry merge
            sc = sb.tile([P, NT], bf16)
            nc.vector.tensor_tensor_scan(
                sc[:], e[:], z[:], 0.0, Alu.add, Alu.add
            )

            nc.sync.dma_start(sc_out[:], sc[:])

    _strip_dead_consts(nc)
    _split_sync_waits(nc, mybir)
    return nc


def _strip_dead_consts(nc):
    """Dead-store elimination: the Bass prologue memsets four const-AP
    tensors (activation bias/scale immediates). This kernel feeds the Exp
    bias from its own zero tile, so none are referenced — but the memsets
    still run serially on Pool ahead of the start barrier, delaying the
    input DMA. Remove any const-* memset whose tensor no other
    instruction touches (semantics-preserving: the tensors stay declared,
    just uninitialized and unread)."""
    used: set = set()
    memsets = []
    for f in nc.m.functions:
        for blk in f.blocks:
            for ins in blk.instructions:
                is_const_memset = False
                if type(ins).__name__ == "InstMemset":
                    ref = getattr(ins.outs[0], "memref", "") or ""
                    if ref.startswith("const-"):
                        memsets.append((blk, ins, ref))
                        is_const_memset = True
                if not is_const_memset:
                    for arg in list(getattr(ins, "ins", [])) + list(
                        getattr(ins, "outs", [])
                    ):
                        ref = getattr(arg, "memref", None)
                        if ref:
                            used.add(ref)
    for blk, ins, ref in memsets:
        if ref not in used:
            blk.instructions.remove(ins)


def _split_sync_waits(nc, mybir, max_waits=1):
    """walrus rejects instructions with too many sync waits. Hoist excess
    waits onto same-engine NoOps inserted immediately before the offending
    instruction — waits execute in order on the engine sequencer, so this
    is equivalent.

    Waits left ON an engine instruction park in its wait queue without
    blocking the sequencer, while NoOp waits stall the sequencer until
    satisfied — so keep the latest-satisfied semaphore threshold on the
    instruction and hoist the early ones."""
    # (sem id, threshold) -> program position of the update that first
    # reaches the threshold (sem-ge-imm waits against sem-inc updates;
    # anything unrecognized pessimistically ranks as "late")
    sem_hist: dict = {}
    pos = 0
    for f in nc.m.functions:
        for blk in f.blocks:
            for ins in blk.instructions:
                si = getattr(ins, "sync_info", None)
                if si:
                    # DMA completion sems post asynchronously, long after
                    # the trigger instruction dispatches — rank them after
                    # every same-program compute update
                    late = (1 << 20) if "DMA" in type(ins).__name__ else 0
                    for u in si.on_update:
                        if u.update_mode == "sem-inc" and u.update_value:
                            tot, hist = sem_hist.setdefault(u.id, [0, []])
                            ntot = tot + u.update_value
                            hist.append((ntot, pos + late))
                            sem_hist[u.id][0] = ntot
                pos += 1

    def satisfier(w):
        """Program position of the update reaching the wait threshold."""
        if w.wait_mode != "sem-ge-imm" or w.id not in sem_hist:
            return 1 << 30
        for tot, p in sem_hist[w.id][1]:
            if tot >= w.wait_value:
                return p
        return 1 << 30

    serial = 0
    for f in nc.m.functions:
        for blk in f.blocks:
            il = blk.instructions
            pos = 0
            while pos < len(il):
                ins = il[pos]
                si = getattr(ins, "sync_info", None)
                if si is None or len(si.on_wait) <= max_waits:
                    pos += 1
                    continue
                waits = sorted(si.on_wait, key=satisfier)
                ins.sync_info = mybir.SyncInfo(
                    on_wait=waits[-max_waits:] if waits else [],
                    on_update=list(si.on_update),
                )
                extra = waits[: -max_waits] if len(waits) > max_waits else []
                for i in range(0, len(extra), max_waits):
                    nop = mybir.InstNoOp(name=f"I-waitsplit-{serial}", ins=[], outs=[])
                    serial += 1
                    nop.engine = ins.engine
                    nop.sync_info = mybir.SyncInfo(
                        on_wait=extra[i : i + max_waits], on_update=[]
                    )
                    nc.register_instruction(nop, overwrite=True)
                    il.insert(pos, nop)
                    pos += 1
                pos += 1


def _get_program():
    if "nc" not in _CACHE:
        _ensure_path()
        _CACHE["nc"] = _build_program()
    return _CACHE["nc"]


def kernel(hazard, time, censor):
    _ensure_path()
    from concourse.bass_utils import run_bass_kernel_spmd

    hazard = np.asarray(hazard, dtype=np.float32)
    time = np.asarray(time, dtype=np.float32)
    censor = np.asarray(censor, dtype=np.float32)

    # descending-time order; ties may land in any order within their group
    pd = np.argsort(-time, kind="stable")
    hs2d = np.ascontiguousarray(hazard[pd].reshape(P, NT).astype(np.float16))

    nc = _get_program()
    in_maps = [{"hs": hs2d} for _ in range(NCORES)]
    res = run_bass_kernel_spmd(nc, in_maps, list(range(NCORES)))
    sc = np.asarray(res.results[0]["sc"], dtype=np.float32)  # bf16 -> fp32

    # S[k = p*64 + t] = within-partition prefix + carry of partitions < p
    # (the 128 partition totals are sc[:, NT-1]; merge their exclusive
    # cumsum on the host, mirroring how multi-core partials would merge)
    carry = np.zeros(P, dtype=np.float32)
    np.cumsum(sc[:-1, NT - 1], dtype=np.float32, out=carry[1:])
    S = (sc + carry[:, None]).reshape(N)

    # cnt_i = |{j : time_j >= time_i}|; risk_sum_i is the prefix at cnt_i-1
    asc = np.sort(time)
    cnt = N - np.searchsorted(asc, time, side="left")
    logrisk = np.log(S[cnt - 1])
    loss = -np.mean((hazard - logrisk) * censor, dtype=np.float32)
    return np.float32(loss)


# revision 26
# speedup vs baseline: 1.0885x; 1.0885x over previous
"""Cox proportional-hazards negative partial log-likelihood on 8 Trainium2
NeuronCores.

reference:
    risk_mask[i, j] = (time[j] >= time[i])
    risk_sum[i]     = sum_j exp(hazard[j]) * risk_mask[i, j]
    loss            = -mean((hazard - log(risk_sum)) * censor)

Because the risk set {j : time_j >= time_i} is a prefix of the
descending-time order, the O(N^2) masked reduction collapses to a prefix
sum: with hazard sorted by time descending,

    S[k]        = sum_{k' <= k} exp(hazard_sorted[k'])
    risk_sum[i] = S[cnt_i - 1],   cnt_i = |{j : time_j >= time_i}|

which is exact under ties (every tie of time_i sits inside the prefix).

Split of work:
  * device (each core): the O(N^2)-collapsed FP reduction — exp(hazard)
    on ACT, then the 8192-long inclusive prefix scan via the DVE
    TensorTensorScanArith recurrence on a [128 partitions x 64] layout
    (k = p*64 + t). One input DMA (fp16 [128, 64]) and one output DMA
    (bf16 [128, 64]) — the kernel is pure stream-through, which is the
    memory-regime shape this problem targets.
  * host: index bookkeeping plus the final O(N) merge — argsort by time,
    searchsorted for cnt_i, the 128-way exclusive cumsum of the
    partition totals (sc[:, -1], the scan's last column) to splice the
    per-partition prefixes into the global S, then log / gather /
    censored mean. This mirrors the previous masked-matmul kernel's host
    role (rank relabeling via np.unique, 8-way partial-sum merge, log,
    mean).
  * sharding: per-core work is O(N) = 16KB streamed, far below the cost
    of any cross-core collective, so the scan is replicated on all 8
    cores (SPMD requires a single program; output-range sharding would
    need per-core programs) and core 0's output is used.
"""

import numpy as np

N = 8192
P = 128
NT = N // P          # 64 elements per partition
NCORES = 8

_CACHE: dict = {}


def _ensure_path():
    try:
        import concourse.bass  # noqa: F401
    except ImportError:
        import sys

        sys.path.insert(0, "/opt/trn_rl_repo")


def _build_program():
    import concourse.bass as bass
    import concourse.mybir as mybir
    from concourse import tile

    f32 = mybir.dt.float32
    f16 = mybir.dt.float16
    bf16 = mybir.dt.bfloat16
    Alu = mybir.AluOpType
    Act = mybir.ActivationFunctionType

    nc = bass.Bass()
    # hazard sorted by time descending, reshaped [128, 64] (k = p*64 + t)
    hs = nc.declare_dram_parameter("hs", [P, NT], f16, isOutput=False)
    # per-partition inclusive prefix sums of exp(hazard)
    sc_out = nc.declare_dram_parameter("sc", [P, NT], bf16, isOutput=True)

    with tile.TileContext(nc) as tc:
        with tc.tile_pool(name="sb", bufs=1) as sb:
            h = sb.tile([P, NT], f16)
            nc.sync.dma_start(h[:], hs[:])
            z = sb.tile([P, NT], f32)
            nc.vector.memset(z[:], 0.0)

            # bias rides the z tile (already zeros) instead of the float
            # default: a float bias forces a const-AP, and with no const-AP
            # users the Bass prologue's four const memsets become dead
            # stores that _strip_dead_consts removes (they serialize on
            # Pool ahead of the start barrier, delaying the input DMA)
            e = sb.tile([P, NT], f32)
            nc.scalar.activation(e[:], h[:], Act.Exp, bias=z[:, 0:1])

            # sc[p, t] = sum_{t' <= t} e[p, t']  (DVE recurrence); the last
            # column carries the per-partition row total for the host-side
            # 128-way carry merge
            sc = sb.tile([P, NT], bf16)
            nc.vector.tensor_tensor_scan(
                sc[:], e[:], z[:], 0.0, Alu.add, Alu.add
            )

            nc.sync.dma_start(sc_out[:], sc[:])

    _hoist_input_dma(nc)
    _strip_dead_consts(nc)
    _split_sync_waits(nc, mybir)
    return nc


def _hoist_input_dma(nc):
    """Start the input DMA before the all-engine start barrier.

    The DMA trigger only needs SP's own register prologue (already done),
    not the other engines' — so move it from the body block to just
    before SP's barrier Drain. The ~2.3us DMA pipeline then overlaps the
    barrier. SP's barrier arrival shifts behind the DMA dispatch, but
    every consumer on the other engines is gated on the DMA-completion
    semaphore (far later), so the later barrier release costs nothing.
    The DMA has no waits and its completion semaphore is only read by
    the post-barrier body, so ordering semantics are unchanged."""
    blocks = [blk for f in nc.m.functions for blk in f.blocks]
    for blk in blocks[1:]:
        for ins in list(blk.instructions):
            if type(ins).__name__ == "InstDMACopy" and "hs" in str(
                getattr(ins.ins[0], "memref", "") or ""
            ):
                si = getattr(ins, "sync_info", None)
                assert si is None or not si.on_wait, "input DMA must be waitless"
                blk.instructions.remove(ins)
                for i, p in enumerate(blocks[0].instructions):
                    if type(p).__name__ == "InstDrain" and p.engine == ins.engine:
                        blocks[0].instructions.insert(i, ins)
                        return
                raise AssertionError("SP barrier Drain not found in preamble")


def _strip_dead_consts(nc):
    """Dead-store elimination: the Bass prologue memsets four const-AP
    tensors (activation bias/scale immediates). This kernel feeds the Exp
    bias from its own zero tile, so none are referenced — but the memsets
    still run serially on Pool ahead of the start barrier, delaying the
    input DMA. Remove any const-* memset whose tensor no other
    instruction touches (semantics-preserving: the tensors stay declared,
    just uninitialized and unread)."""
    used: set = set()
    memsets = []
    for f in nc.m.functions:
        for blk in f.blocks:
            for ins in blk.instructions:
                is_const_memset = False
                if type(ins).__name__ == "InstMemset":
                    ref = getattr(ins.outs[0], "memref", "") or ""
                    if ref.startswith("const-"):
                        memsets.append((blk, ins, ref))
                        is_const_memset = True
                if not is_const_memset:
                    for arg in list(getattr(ins, "ins", [])) + list(
                        getattr(ins, "outs", [])
                    ):
                        ref = getattr(arg, "memref", None)
                        if ref:
                            used.add(ref)
    for blk, ins, ref in memsets:
        if ref not in used:
            blk.instructions.remove(ins)


def _split_sync_waits(nc, mybir, max_waits=1):
    """walrus rejects instructions with too many sync waits. Hoist excess
    waits onto same-engine NoOps inserted immediately before the offending
    instruction — waits execute in order on the engine sequencer, so this
    is equivalent.

    Waits left ON an engine instruction park in its wait queue without
    blocking the sequencer, while NoOp waits stall the sequencer until
    satisfied — so keep the latest-satisfied semaphore threshold on the
    instruction and hoist the early ones."""
    # (sem id, threshold) -> program position of the update that first
    # reaches the threshold (sem-ge-imm waits against sem-inc updates;
    # anything unrecognized pessimistically ranks as "late")
    sem_hist: dict = {}
    pos = 0
    for f in nc.m.functions:
        for blk in f.blocks:
            for ins in blk.instructions:
                si = getattr(ins, "sync_info", None)
                if si:
                    # DMA completion sems post asynchronously, long after
                    # the trigger instruction dispatches — rank them after
                    # every same-program compute update
                    late = (1 << 20) if "DMA" in type(ins).__name__ else 0
                    for u in si.on_update:
                        if u.update_mode == "sem-inc" and u.update_value:
                            tot, hist = sem_hist.setdefault(u.id, [0, []])
                            ntot = tot + u.update_value
                            hist.append((ntot, pos + late))
                            sem_hist[u.id][0] = ntot
                pos += 1

    def satisfier(w):
        """Program position of the update reaching the wait threshold."""
        if w.wait_mode != "sem-ge-imm" or w.id not in sem_hist:
            return 1 << 30
        for tot, p in sem_hist[w.id][1]:
            if tot >= w.wait_value:
                return p
        return 1 << 30

    serial = 0
    for f in nc.m.functions:
        for blk in f.blocks:
            il = blk.instructions
            pos = 0
            while pos < len(il):
                ins = il[pos]
                si = getattr(ins, "sync_info", None)
                if si is None or len(si.on_wait) <= max_waits:
                    pos += 1
                    continue
                waits = sorted(si.on_wait, key=satisfier)
                ins.sync_info = mybir.SyncInfo(
                    on_wait=waits[-max_waits:] if waits else [],
                    on_update=list(si.on_update),
                )
                extra = waits[: -max_waits] if len(waits) > max_waits else []
                for i in range(0, len(extra), max_waits):
                    nop = mybir.InstNoOp(name=f"I-waitsplit-{serial}", ins=[], outs=[])
                    serial += 1
                    nop.engine = ins.engine
                    nop.sync_info = mybir.SyncInfo(
                        on_wait=extra[i : i + max_waits], on_update=[]
                    )
                    nc.register_instruction(nop, overwrite=True)
                    il.insert(pos, nop)
                    pos += 1
                pos += 1


def _get_program():
    if "nc" not in _CACHE:
        _ensure_path()
        _CACHE["nc"] = _build_program()
    return _CACHE["nc"]


def kernel(hazard, time, censor):
    _ensure_path()
    from concourse.bass_utils import run_bass_kernel_spmd

    hazard = np.asarray(hazard, dtype=np.float32)
    time = np.asarray(time, dtype=np.float32)
    censor = np.asarray(censor, dtype=np.float32)

    # descending-time order; ties may land in any order within their group
    pd = np.argsort(-time, kind="stable")
    hs2d = np.ascontiguousarray(hazard[pd].reshape(P, NT).astype(np.float16))

    nc = _get_program()
    in_maps = [{"hs": hs2d} for _ in range(NCORES)]
    res = run_bass_kernel_spmd(nc, in_maps, list(range(NCORES)))
    sc = np.asarray(res.results[0]["sc"], dtype=np.float32)  # bf16 -> fp32

    # S[k = p*64 + t] = within-partition prefix + carry of partitions < p
    # (the 128 partition totals are sc[:, NT-1]; merge their exclusive
    # cumsum on the host, mirroring how multi-core partials would merge)
    carry = np.zeros(P, dtype=np.float32)
    np.cumsum(sc[:-1, NT - 1], dtype=np.float32, out=carry[1:])
    S = (sc + carry[:, None]).reshape(N)

    # cnt_i = |{j : time_j >= time_i}|; risk_sum_i is the prefix at cnt_i-1
    asc = np.sort(time)
    cnt = N - np.searchsorted(asc, time, side="left")
    logrisk = np.log(S[cnt - 1])
    loss = -np.mean((hazard - logrisk) * censor, dtype=np.float32)
    return np.float32(loss)


# revision 28
# speedup vs baseline: 1.1155x; 1.0249x over previous
"""Cox proportional-hazards negative partial log-likelihood on 8 Trainium2
NeuronCores.

reference:
    risk_mask[i, j] = (time[j] >= time[i])
    risk_sum[i]     = sum_j exp(hazard[j]) * risk_mask[i, j]
    loss            = -mean((hazard - log(risk_sum)) * censor)

Because the risk set {j : time_j >= time_i} is a prefix of the
descending-time order, the O(N^2) masked reduction collapses to a prefix
sum: with hazard sorted by time descending,

    S[k]        = sum_{k' <= k} exp(hazard_sorted[k'])
    risk_sum[i] = S[cnt_i - 1],   cnt_i = |{j : time_j >= time_i}|

which is exact under ties (every tie of time_i sits inside the prefix).

Split of work:
  * device (each core): the O(N^2)-collapsed FP reduction — exp(hazard)
    on ACT, then the 8192-long inclusive prefix scan via the DVE
    TensorTensorScanArith recurrence on a [128 partitions x 64] layout
    (k = p*64 + t). One input DMA (fp16 [128, 64]) and one output DMA
    (bf16 [128, 64]) — the kernel is pure stream-through, which is the
    memory-regime shape this problem targets.
  * host: index bookkeeping plus the final O(N) merge — argsort by time,
    searchsorted for cnt_i, the 128-way exclusive cumsum of the
    partition totals (sc[:, -1], the scan's last column) to splice the
    per-partition prefixes into the global S, then log / gather /
    censored mean. This mirrors the previous masked-matmul kernel's host
    role (rank relabeling via np.unique, 8-way partial-sum merge, log,
    mean).
  * sharding: per-core work is O(N) = 16KB streamed, far below the cost
    of any cross-core collective, so the scan is replicated on all 8
    cores (SPMD requires a single program; output-range sharding would
    need per-core programs) and core 0's output is used.
"""

import numpy as np

N = 8192
P = 128
NT = N // P          # 64 elements per partition
NCORES = 8

_CACHE: dict = {}


def _ensure_path():
    try:
        import concourse.bass  # noqa: F401
    except ImportError:
        import sys

        sys.path.insert(0, "/opt/trn_rl_repo")


def _build_program():
    import concourse.bass as bass
    import concourse.mybir as mybir
    from concourse import tile

    f32 = mybir.dt.float32
    f16 = mybir.dt.float16
    bf16 = mybir.dt.bfloat16
    Alu = mybir.AluOpType
    Act = mybir.ActivationFunctionType

    nc = bass.Bass()
    # hazard sorted by time descending, reshaped [128, 64] (k = p*64 + t)
    hs = nc.declare_dram_parameter("hs", [P, NT], f16, isOutput=False)
    # per-partition inclusive prefix sums of exp(hazard)
    sc_out = nc.declare_dram_parameter("sc", [P, NT], bf16, isOutput=True)

    with tile.TileContext(nc) as tc:
        with tc.tile_pool(name="sb", bufs=1) as sb:
            h = sb.tile([P, NT], f16)
            nc.sync.dma_start(h[:], hs[:])
            z = sb.tile([P, NT], f32)
            nc.vector.memset(z[:], 0.0)

            # bias rides the z tile (already zeros) instead of the float
            # default: a float bias forces a const-AP, and with no const-AP
            # users the Bass prologue's four const memsets become dead
            # stores that _strip_dead_consts removes (they serialize on
            # Pool ahead of the start barrier, delaying the input DMA)
            e = sb.tile([P, NT], f32)
            nc.scalar.activation(e[:], h[:], Act.Exp, bias=z[:, 0:1])

            # sc[p, t] = sum_{t' <= t} e[p, t']  (DVE recurrence); the last
            # column carries the per-partition row total for the host-side
            # 128-way carry merge
            sc = sb.tile([P, NT], bf16)
            nc.vector.tensor_tensor_scan(
                sc[:], e[:], z[:], 0.0, Alu.add, Alu.add
            )

            nc.sync.dma_start(sc_out[:], sc[:])

    _hoist_input_dma(nc)
    _defer_dma_wait_to_clear(nc, mybir)
    _strip_dead_consts(nc)
    _split_sync_waits(nc, mybir)
    return nc


def _defer_dma_wait_to_clear(nc, mybir):
    """Overlap the teardown barrier rounds with the output DMA's 900ns
    completion-semaphore window.

    The teardown is: [SP drain waits out-DMA sem] -> barrier round 1 ->
    Pool EVENT_SEMAPHORE_RANGE_CLEAR -> barrier round 2. The clear only
    has to follow the last DATA-semaphore update (the out-DMA's +16);
    barrier semaphores are excluded from clears (round 2's own barrier
    updates post after the clear, which would otherwise corrupt it). So
    move the DMA wait off SP's round-1 drain onto Pool, immediately
    before the clear: both barrier rounds then run during the DMA wait,
    and program end still observes the output landing."""
    blocks = [blk for f in nc.m.functions for blk in f.blocks]
    tail = blocks[-1].instructions
    dma_wait = None
    for ins in tail:
        si = getattr(ins, "sync_info", None)
        if (
            type(ins).__name__ == "InstDrain"
            and si
            and any("DMAHW" in w.ant_name for w in si.on_wait)
        ):
            dma_wait = [w for w in si.on_wait if "DMAHW" in w.ant_name]
            ins.sync_info = mybir.SyncInfo(
                on_wait=[w for w in si.on_wait if "DMAHW" not in w.ant_name],
                on_update=list(si.on_update),
            )
            break
    assert dma_wait, "teardown DMA-completion drain not found"
    for i, ins in enumerate(tail):
        if type(ins).__name__ == "InstISA":
            for j, w in enumerate(dma_wait):
                nop = mybir.InstNoOp(
                    name=f"I-dma-wait-pre-clear-{j}", ins=[], outs=[]
                )
                nop.engine = ins.engine
                nop.sync_info = mybir.SyncInfo(on_wait=[w], on_update=[])
                nc.register_instruction(nop, overwrite=True)
                tail.insert(i + j, nop)
            return
    raise AssertionError("teardown clear (InstISA) not found")


def _hoist_input_dma(nc):
    """Start the input DMA before the all-engine start barrier.

    The DMA trigger only needs SP's own register prologue (already done),
    not the other engines' — so move it from the body block to just
    before SP's barrier Drain. The ~2.3us DMA pipeline then overlaps the
    barrier. SP's barrier arrival shifts behind the DMA dispatch, but
    every consumer on the other engines is gated on the DMA-completion
    semaphore (far later), so the later barrier release costs nothing.
    The DMA has no waits and its completion semaphore is only read by
    the post-barrier body, so ordering semantics are unchanged."""
    blocks = [blk for f in nc.m.functions for blk in f.blocks]
    for blk in blocks[1:]:
        for ins in list(blk.instructions):
            if type(ins).__name__ == "InstDMACopy" and "hs" in str(
                getattr(ins.ins[0], "memref", "") or ""
            ):
                si = getattr(ins, "sync_info", None)
                assert si is None or not si.on_wait, "input DMA must be waitless"
                blk.instructions.remove(ins)
                for i, p in enumerate(blocks[0].instructions):
                    if type(p).__name__ == "InstDrain" and p.engine == ins.engine:
                        blocks[0].instructions.insert(i, ins)
                        return
                raise AssertionError("SP barrier Drain not found in preamble")


def _strip_dead_consts(nc):
    """Dead-store elimination: the Bass prologue memsets four const-AP
    tensors (activation bias/scale immediates). This kernel feeds the Exp
    bias from its own zero tile, so none are referenced — but the memsets
    still run serially on Pool ahead of the start barrier, delaying the
    input DMA. Remove any const-* memset whose tensor no other
    instruction touches (semantics-preserving: the tensors stay declared,
    just uninitialized and unread)."""
    used: set = set()
    memsets = []
    for f in nc.m.functions:
        for blk in f.blocks:
            for ins in blk.instructions:
                is_const_memset = False
                if type(ins).__name__ == "InstMemset":
                    ref = getattr(ins.outs[0], "memref", "") or ""
                    if ref.startswith("const-"):
                        memsets.append((blk, ins, ref))
                        is_const_memset = True
                if not is_const_memset:
                    for arg in list(getattr(ins, "ins", [])) + list(
                        getattr(ins, "outs", [])
                    ):
                        ref = getattr(arg, "memref", None)
                        if ref:
                            used.add(ref)
    for blk, ins, ref in memsets:
        if ref not in used:
            blk.instructions.remove(ins)


def _split_sync_waits(nc, mybir, max_waits=1):
    """walrus rejects instructions with too many sync waits. Hoist excess
    waits onto same-engine NoOps inserted immediately before the offending
    instruction — waits execute in order on the engine sequencer, so this
    is equivalent.

    Waits left ON an engine instruction park in its wait queue without
    blocking the sequencer, while NoOp waits stall the sequencer until
    satisfied — so keep the latest-satisfied semaphore threshold on the
    instruction and hoist the early ones."""
    # (sem id, threshold) -> program position of the update that first
    # reaches the threshold (sem-ge-imm waits against sem-inc updates;
    # anything unrecognized pessimistically ranks as "late")
    sem_hist: dict = {}
    pos = 0
    for f in nc.m.functions:
        for blk in f.blocks:
            for ins in blk.instructions:
                si = getattr(ins, "sync_info", None)
                if si:
                    # DMA completion sems post asynchronously, long after
                    # the trigger instruction dispatches — rank them after
                    # every same-program compute update
                    late = (1 << 20) if "DMA" in type(ins).__name__ else 0
                    for u in si.on_update:
                        if u.update_mode == "sem-inc" and u.update_value:
                            tot, hist = sem_hist.setdefault(u.id, [0, []])
                            ntot = tot + u.update_value
                            hist.append((ntot, pos + late))
                            sem_hist[u.id][0] = ntot
                pos += 1

    def satisfier(w):
        """Program position of the update reaching the wait threshold."""
        if w.wait_mode != "sem-ge-imm" or w.id not in sem_hist:
            return 1 << 30
        for tot, p in sem_hist[w.id][1]:
            if tot >= w.wait_value:
                return p
        return 1 << 30

    serial = 0
    for f in nc.m.functions:
        for blk in f.blocks:
            il = blk.instructions
            pos = 0
            while pos < len(il):
                ins = il[pos]
                si = getattr(ins, "sync_info", None)
                if si is None or len(si.on_wait) <= max_waits:
                    pos += 1
                    continue
                waits = sorted(si.on_wait, key=satisfier)
                ins.sync_info = mybir.SyncInfo(
                    on_wait=waits[-max_waits:] if waits else [],
                    on_update=list(si.on_update),
                )
                extra = waits[: -max_waits] if len(waits) > max_waits else []
                for i in range(0, len(extra), max_waits):
                    nop = mybir.InstNoOp(name=f"I-waitsplit-{serial}", ins=[], outs=[])
                    serial += 1
                    nop.engine = ins.engine
                    nop.sync_info = mybir.SyncInfo(
                        on_wait=extra[i : i + max_waits], on_update=[]
                    )
                    nc.register_instruction(nop, overwrite=True)
                    il.insert(pos, nop)
                    pos += 1
                pos += 1


def _get_program():
    if "nc" not in _CACHE:
        _ensure_path()
        _CACHE["nc"] = _build_program()
    return _CACHE["nc"]


def kernel(hazard, time, censor):
    _ensure_path()
    from concourse.bass_utils import run_bass_kernel_spmd

    hazard = np.asarray(hazard, dtype=np.float32)
    time = np.asarray(time, dtype=np.float32)
    censor = np.asarray(censor, dtype=np.float32)

    # descending-time order; ties may land in any order within their group
    pd = np.argsort(-time, kind="stable")
    hs2d = np.ascontiguousarray(hazard[pd].reshape(P, NT).astype(np.float16))

    nc = _get_program()
    in_maps = [{"hs": hs2d} for _ in range(NCORES)]
    res = run_bass_kernel_spmd(nc, in_maps, list(range(NCORES)))
    sc = np.asarray(res.results[0]["sc"], dtype=np.float32)  # bf16 -> fp32

    # S[k = p*64 + t] = within-partition prefix + carry of partitions < p
    # (the 128 partition totals are sc[:, NT-1]; merge their exclusive
    # cumsum on the host, mirroring how multi-core partials would merge)
    carry = np.zeros(P, dtype=np.float32)
    np.cumsum(sc[:-1, NT - 1], dtype=np.float32, out=carry[1:])
    S = (sc + carry[:, None]).reshape(N)

    # cnt_i = |{j : time_j >= time_i}|; risk_sum_i is the prefix at cnt_i-1
    asc = np.sort(time)
    cnt = N - np.searchsorted(asc, time, side="left")
    logrisk = np.log(S[cnt - 1])
    loss = -np.mean((hazard - logrisk) * censor, dtype=np.float32)
    return np.float32(loss)
